# revision 10
# baseline (speedup 1.0000x reference)
"""Trainium2 Bass kernel for nn_BasicGroupCNN12 (SE(2) group CNN).

Strategy (8 NeuronCores):
  - Shard: 2 batch images x 4 spatial quadrants (64x64) = 8 cores.
  - Each core computes its quadrant with a shrinking redundant margin
    (lift needs +18, each 5x5 conv consumes 2) -> no halo exchange.
  - Host flips each quadrant (and the conv kernels) so every core runs the
    IDENTICAL program with the true image corner at local (0,0); reflect
    padding is then a uniform local copy on the low sides.
  - Training-mode BatchNorm needs global batch stats per layer: per-chunk
    bn_stats on the PSUM conv output (one DVE pass), bn_aggr to a local
    (mean, E[y^2]) pair, then an 8-core AllGather (floor ~5us vs ~10us for
    AllReduce) + local rank-sum + an orientation group-mean PE matmul.
    The margin chunks run on the PE while the collective is in flight.
  - Convs are per-offset matmuls: K=96 (ci*orient zero-padded from 80),
    M=80 (orient*co), N<=512 pixels, accumulated in PSUM over 25 offsets.
  - Activations live in two static ping-pong SBUF buffers [96,84,84];
    K-pad partitions 80:96 are zeroed once at startup (no per-layer memset).
  - Matmul operands fp16 (PSUM accumulation fp32): ~6e-4 final rel err.
"""

import os
import numpy as np

import concourse.bass as bass
import concourse.mybir as mybir
import concourse.tile as tile
from concourse import bacc
from concourse import bass_utils

F16 = mybir.dt.float16
F32 = mybir.dt.float32
NPDT = np.float16

O = 8        # orientations
C = 10       # channels per orientation (conv layers)
CF = 16      # channels in layer 10
E0 = 82      # lift output extent per core
EPS = 1e-5
QN = 64 * 64                 # in-image pixels per core
AF = mybir.ActivationFunctionType
ALU = mybir.AluOpType

# conv_i (i=1..9) output extents per core
CONV_E = [64 + 2 * (9 - i) for i in range(1, 10)]   # 80,78,...,64


# ---------------------------------------------------------------------------
# Host-side weight/input preparation (pure numpy)
# ---------------------------------------------------------------------------

def _rot_matrices(k, n):
    c = (k - 1) / 2.0
    mats = np.zeros((n, k * k, k * k), np.float32)
    for m in range(n):
        th = 2.0 * np.pi * m / n
        co, si = np.cos(th), np.sin(th)
        for i in range(k):
            for j in range(k):
                di, dj = i - c, j - c
                sy = co * di + si * dj + c
                sx = -si * di + co * dj + c
                i0, j0 = int(np.floor(sy)), int(np.floor(sx))
                fy, fx = sy - i0, sx - j0
                for ii, jj, w in ((i0, j0, (1 - fy) * (1 - fx)), (i0, j0 + 1, (1 - fy) * fx),
                                  (i0 + 1, j0, fy * (1 - fx)), (i0 + 1, j0 + 1, fy * fx)):
                    if 0 <= ii < k and 0 <= jj < k and w > 1e-8:
                        mats[m, i * k + j, ii * k + jj] = w
    return mats


def _prep_rotated_weights(lift_w, conv_w, w10):
    M7 = _rot_matrices(7, O)
    M5 = _rot_matrices(5, O)
    Wlift = np.zeros((O, C, 3, 7, 7), np.float32)          # [m,co,ci,dy,dx]
    for m in range(O):
        Wlift[m] = (lift_w.reshape(C, 3, 49) @ M7[m].T).reshape(C, 3, 7, 7)
    Wconv = []
    for L in range(9):
        W = conv_w[L]                                      # [co,ci,n,5,5]
        Wf = np.zeros((O, C, O, C, 5, 5), np.float32)      # [m,co,n,ci,dy,dx]
        for m in range(O):
            Wm = np.roll(W, m, axis=2)
            Wm = (Wm.reshape(C, C, O, 25) @ M5[m].T).reshape(C, C, O, 5, 5)
            Wf[m] = Wm.transpose(0, 2, 1, 3, 4)
        Wconv.append(Wf)
    # w10 [16,10,8,1,1] -> mat [K=n*10+ci, M=m*16+co]; K zero-padded to 96
    # (K=80 matmuls stream at half rate on TRN2; K>=96 at full rate)
    W10mat = np.zeros((96, 128), np.float32)
    for m in range(O):
        Wm = np.roll(w10[:, :, :, 0, 0], m, axis=2)        # [co,ci,n]
        W10mat[:80, m * CF:(m + 1) * CF] = Wm.transpose(2, 1, 0).reshape(80, CF)
    return Wlift, Wconv, W10mat


def _group_mat(P, nch):
    # orientation group-MEAN matrix: the collective carries per-partition
    # (mean, E[y^2]) local averages; merging 8 orientation partitions x 8
    # cores (all with equal counts) is a plain average -> 1/64.
    idx = np.arange(P)
    Gm = (idx[:, None] % nch == idx[None, :] % nch).astype(np.float32)
    return Gm * (1.0 / 64.0)


def prep_in_maps(inputs):
    x = np.asarray(inputs['x'], np.float32)
    lift_w = np.asarray(inputs['lift_w'], np.float32)
    conv_w = np.asarray(inputs['conv_w'], np.float32)
    w10 = np.asarray(inputs['w10'], np.float32)
    wfinal = np.asarray(inputs['wfinal'], np.float32)

    Wlift, Wconv, W10mat = _prep_rotated_weights(lift_w, conv_w, w10)

    gb = np.zeros((80, 10, 2), np.float32)
    gb[:, 0, 0] = np.asarray(inputs['lift_g'], np.float32)[np.arange(80) % C]
    gb[:, 0, 1] = np.asarray(inputs['lift_b'], np.float32)[np.arange(80) % C]
    cg = np.asarray(inputs['conv_g'], np.float32)
    cb = np.asarray(inputs['conv_b'], np.float32)
    for L in range(9):
        gb[:, L + 1, 0] = cg[L][np.arange(80) % C]
        gb[:, L + 1, 1] = cb[L][np.arange(80) % C]
    gb10 = np.zeros((128, 2), np.float32)
    gb10[:, 0] = np.asarray(inputs['g10'], np.float32)[np.arange(128) % CF]
    gb10[:, 1] = np.asarray(inputs['b10'], np.float32)[np.arange(128) % CF]

    G = _group_mat(80, C)
    G10 = _group_mat(128, CF)
    wfin = wfinal[0, :, 0, 0].reshape(16, 1).astype(NPDT)

    # per-(qy,qx) flipped weight variants
    wvar = {}
    for qy in range(2):
        for qx in range(2):
            fy = slice(None, None, -1) if qy else slice(None)
            fx = slice(None, None, -1) if qx else slice(None)
            Wl = Wlift[:, :, :, fy, fx]                    # [m,co,ci,dy,dx]
            liftWmat = np.zeros((192, 80), np.float32)     # 147 rows + K-pad
            liftWmat[:147] = (
                Wl.transpose(3, 4, 2, 0, 1).reshape(49, 3, 80).reshape(147, 80))
            Wc = np.zeros((96, 9, 25, 80), np.float32)     # [K(padded), L, d, M]
            for L in range(9):
                Wf = Wconv[L][:, :, :, :, fy, fx]          # [m,co,n,ci,dy,dx]
                Wd = Wf.transpose(4, 5, 2, 3, 0, 1).reshape(25, 80, 80)
                Wc[:80, L] = Wd.transpose(1, 0, 2)         # [K, d, M]
            wvar[(qy, qx)] = (liftWmat.astype(NPDT), Wc.astype(NPDT))

    in_maps = []
    cores = []
    for b in range(2):
        for qy in range(2):
            for qx in range(2):
                cores.append((b, qy, qx))
                xf = x[b]
                if qy:
                    xf = xf[:, ::-1, :]
                if qx:
                    xf = xf[:, :, ::-1]
                win = xf[:, 0:85, 0:85]
                xw = np.pad(win, ((0, 0), (3, 0), (3, 0)), mode='reflect')
                col = np.zeros((192, E0 * E0), NPDT)       # 147 rows + K-pad
                for t in range(49):
                    dy, dx = t // 7, t % 7
                    col[t * 3:t * 3 + 3] = (
                        xw[:, dy:dy + E0, dx:dx + E0].reshape(3, -1))
                liftWmat, Wc = wvar[(qy, qx)]
                in_maps.append({
                    'xcol1': np.ascontiguousarray(col[:96]),
                    'xcol2': np.ascontiguousarray(col[96:]),
                    'liftW1': np.ascontiguousarray(liftWmat[:96]),
                    'liftW2': np.ascontiguousarray(liftWmat[96:]),
                    'convW': Wc,
                    'w10': W10mat.astype(NPDT),
                    'wfin': wfin,
                    'G': G,
                    'G10': G10,
                    'gb': gb,
                    'gb10': gb10,
                })
    return in_maps, cores


# ---------------------------------------------------------------------------
# Bass program
# ---------------------------------------------------------------------------

def build_program(no_cc=None, nconv=None, stage=None, fold_max=True):
    # bisection knobs (env override): GK_NO_CC=1 replaces the AllGather with
    # local DRAM->DRAM copies; GK_NCONV=k emits only the first k conv layers;
    # GK_STAGE stops the program early.
    if no_cc is None:
        no_cc = bool(os.environ.get("GK_NO_CC"))
    if nconv is None:
        nconv = int(os.environ.get("GK_NCONV", "9"))
    if stage is None:
        stage = int(os.environ.get("GK_STAGE", "6"))
    nc = bacc.Bacc("TRN2", target_bir_lowering=False, debug=False,
                   enable_asserts=False, num_devices=8)

    d_xcol1 = nc.dram_tensor("xcol1", [96, E0 * E0], F16, kind="ExternalInput")
    d_xcol2 = nc.dram_tensor("xcol2", [96, E0 * E0], F16, kind="ExternalInput")
    d_liftW1 = nc.dram_tensor("liftW1", [96, 80], F16, kind="ExternalInput")
    d_liftW2 = nc.dram_tensor("liftW2", [96, 80], F16, kind="ExternalInput")
    d_convW = nc.dram_tensor("convW", [96, 9, 25, 80], F16, kind="ExternalInput")
    d_w10 = nc.dram_tensor("w10", [96, 128], F16, kind="ExternalInput")
    d_wfin = nc.dram_tensor("wfin", [16, 1], F16, kind="ExternalInput")
    d_G = nc.dram_tensor("G", [80, 80], F32, kind="ExternalInput")
    d_G10 = nc.dram_tensor("G10", [128, 128], F32, kind="ExternalInput")
    d_gb = nc.dram_tensor("gb", [80, 10, 2], F32, kind="ExternalInput")
    d_gb10 = nc.dram_tensor("gb10", [128, 2], F32, kind="ExternalInput")
    d_out = nc.dram_tensor("out", [1, 64 * 64], F32, kind="ExternalOutput")

    rg = [list(range(8))]

    with tile.TileContext(nc) as tc:
        with (
            tc.tile_pool(name="const", bufs=1) as constp,
            tc.tile_pool(name="tails", bufs=1) as tailsp,
            tc.tile_pool(name="smallp", bufs=3) as smallp,
            tc.tile_pool(name="pscv", bufs=6, space="PSUM") as pscv,
            tc.tile_pool(name="psst", bufs=2, space="PSUM") as psst,
            tc.tile_pool(name="dram", bufs=2, space="DRAM") as dramp,
        ):
            # ---- load constants ----
            xc1 = constp.tile([96, E0 * E0], F16, tag="xc1")
            nc.sync.dma_start(out=xc1[:], in_=d_xcol1.ap())
            xc2 = constp.tile([96, E0 * E0], F16, tag="xc2")
            nc.sync.dma_start(out=xc2[:], in_=d_xcol2.ap())
            lw1 = constp.tile([96, 80], F16, tag="lw1")
            nc.sync.dma_start(out=lw1[:], in_=d_liftW1.ap())
            lw2 = constp.tile([96, 80], F16, tag="lw2")
            nc.sync.dma_start(out=lw2[:], in_=d_liftW2.ap())
            cw = []
            for L in range(9):
                t = constp.tile([96, 25, 80], F16, tag=f"cw{L}")
                nc.sync.dma_start(out=t[:], in_=d_convW.ap()[:, L, :, :])
                cw.append(t)
            w10t = constp.tile([96, 128], F16, tag="w10")
            nc.sync.dma_start(out=w10t[:], in_=d_w10.ap())
            wfint = constp.tile([16, 1], F16, tag="wfin")
            nc.sync.dma_start(out=wfint[:], in_=d_wfin.ap())
            Gt = constp.tile([80, 80], F32, tag="G")
            nc.sync.dma_start(out=Gt[:], in_=d_G.ap())
            G10t = constp.tile([128, 128], F32, tag="G10")
            nc.sync.dma_start(out=G10t[:], in_=d_G10.ap())
            gbt = constp.tile([80, 10, 2], F32, tag="gb")
            nc.sync.dma_start(out=gbt[:], in_=d_gb.ap())
            gb10t = constp.tile([128, 2], F32, tag="gb10")
            nc.sync.dma_start(out=gb10t[:], in_=d_gb10.ap())
            epst = constp.tile([128, 1], F32, tag="eps")
            nc.vector.memset(epst[:], EPS)

            # ---- static ping-pong activation buffers; K-pad partitions
            # 80:96 hold zeros for the whole program (written once).
            bufA = constp.tile([96, 84, 84], F16, tag="bufA")
            bufB = constp.tile([96, 84, 84], F16, tag="bufB")
            nc.vector.memset(bufA[64:96, :, :], 0.0)
            nc.vector.memset(bufB[64:96, :, :], 0.0)

            def emit_collective(cin, cout):
                if no_cc:
                    for r in range(8):
                        nc.sync.dma_start(out=cout[r, :, :], in_=cin[:])
                else:
                    nc.gpsimd.collective_compute(
                        "AllGather", ALU.bypass, replica_groups=rg,
                        ins=[cin[:].opt()], outs=[cout[:].opt()])

            # warm-up collective (absorbs ncfw first-call latency)
            wu_in = dramp.tile([8, 1], F32, tag="cc_in")
            wu_out = dramp.tile([8, 8, 1], F32, tag="cc_out")
            nc.sync.dma_start(out=wu_in[:], in_=epst[0:8, 0:1])
            emit_collective(wu_in, wu_out)

            def emit_layer(mms_fn, dst, doff, E_out, P, Gtile, gamma_ap,
                           beta_ap, do_reflect, tail_fn=None, do_apply=True):
                """One conv-like layer with overlapped BN.

                Per img chunk: matmuls into PSUM, ACT copy PSUM->SBUF, and a
                one-pass DVE bn_stats on the owned region of the PSUM tile.
                bn_aggr merges the row 6-tuples into local (mean, var); the
                (mean, E[y^2]) pair rides an 8-core AllGather whose latency
                the margin chunks hide; post-collective: rank-sum, group-mean
                matmul, scale/bias chain, relu-apply in two row pieces.
                """
                r = 512 // E_out
                chunks = [(y0, min(r, E_out - y0)) for y0 in range(0, E_out, r)]
                img_chunks = [c for c in chunks if c[0] < 64]
                mar_chunks = [c for c in chunks if c[0] >= 64]
                bst = smallp.tile([P, 64, 6], F32, tag="bst")

                def do_chunk(y0, rr, stats):
                    ps = pscv.tile([P, rr, E_out], F32, tag="cv")
                    mms_fn(ps, y0, rr)
                    nc.scalar.copy(
                        dst[0:P, doff + y0:doff + y0 + rr,
                            doff:doff + E_out], ps[:])
                    if stats:
                        h = min(y0 + rr, 64) - y0
                        for k in range(h):
                            nc.vector.bn_stats(bst[:, y0 + k, :],
                                               ps[0:P, k, 0:64])

                for (y0, rr) in img_chunks:
                    do_chunk(y0, rr, True)
                # local (mean, var) over the owned 64x64, then (mean, E[y^2])
                with tc.high_priority():
                    mv = smallp.tile([P, 2], F32, tag="mv")
                    nc.vector.bn_aggr(
                        mv[:], bst[:].rearrange("p a b -> p (a b)"))
                    v = smallp.tile([P, 2], F32, tag="v")
                    nc.vector.tensor_copy(v[:, 0:1], mv[:, 0:1])
                    msq0 = smallp.tile([P, 1], F32, tag="msq0")
                    nc.vector.tensor_mul(msq0[:], mv[:, 0:1], mv[:, 0:1])
                    nc.vector.tensor_add(v[:, 1:2], msq0[:], mv[:, 1:2])
                    cin = dramp.tile([P, 2], F32, tag="cc_in")
                    cout = dramp.tile([8, P, 2], F32, tag="cc_out")
                    nc.sync.dma_start(out=cin[:], in_=v[:])
                    emit_collective(cin, cout)
                # margin chunks run on the PE while the AllGather is in flight
                for (y0, rr) in mar_chunks:
                    do_chunk(y0, rr, False)
                if tail_fn is not None:
                    tail_fn()
                post = smallp.tile([P, 2, 8], F32, tag="post")
                nc.sync.dma_start(out=post[:],
                                  in_=cout[:].rearrange("r p s -> p s r"))
                m8 = smallp.tile([P, 2], F32, tag="m8")
                nc.vector.tensor_reduce(out=m8[:], in_=post[:],
                                        axis=mybir.AxisListType.X, op=ALU.add)
                # group-mean over orientations+cores (PE; Gtile = mask/64)
                gps = psst.tile([P, 2], F32, tag="st")
                nc.tensor.matmul(gps[:], Gtile[:], m8[:],
                                 start=True, stop=True)
                # var = E[y^2] - mean^2
                msq = smallp.tile([P, 1], F32, tag="msq")
                nc.scalar.square(msq[:], gps[:, 0:1])
                var = smallp.tile([P, 1], F32, tag="var")
                nc.vector.tensor_sub(var[:], gps[:, 1:2], msq[:])
                std = smallp.tile([P, 1], F32, tag="std")
                nc.scalar.activation(std[:], var[:], AF.Sqrt,
                                     bias=epst[0:P, 0:1])
                rstd = smallp.tile([P, 1], F32, tag="rstd")
                nc.vector.reciprocal(rstd[:], std[:])
                scl = smallp.tile([P, 1], F32, tag="scl")
                nc.vector.tensor_mul(scl[:], rstd[:], gamma_ap)
                tb = smallp.tile([P, 1], F32, tag="tb")
                nc.vector.tensor_mul(tb[:], gps[:, 0:1], scl[:])
                bia = smallp.tile([P, 1], F32, tag="bia")
                nc.vector.tensor_sub(bia[:], beta_ap, tb[:])
                if not do_apply:
                    return scl, bia
                # apply relu(y*scl + bia) in place: piece A covers the rows the
                # next layer's first chunk needs; reflect copies ride the ACT
                # queue (no cross-engine sync on the critical path).
                ya = min(12, E_out)
                H = E_out + 2
                pa = dst[0:P, doff:doff + ya, doff:doff + E_out]
                nc.scalar.activation(pa, pa, AF.Relu, bias=bia[:], scale=scl[:])
                if do_reflect:
                    nc.scalar.copy(dst[0:P, 2:2 + ya, 0:1],
                                   dst[0:P, 2:2 + ya, 4:5])
                    nc.scalar.copy(dst[0:P, 2:2 + ya, 1:2],
                                   dst[0:P, 2:2 + ya, 3:4])
                    nc.scalar.copy(dst[0:P, 0:1, 0:H], dst[0:P, 4:5, 0:H])
                    nc.scalar.copy(dst[0:P, 1:2, 0:H], dst[0:P, 3:4, 0:H])
                if ya < E_out:
                    pb = dst[0:P, doff + ya:doff + E_out, doff:doff + E_out]
                    nc.scalar.activation(pb, pb, AF.Relu, bias=bia[:],
                                         scale=scl[:])
                    if do_reflect:
                        nc.scalar.copy(dst[0:P, 2 + ya:H, 0:1],
                                       dst[0:P, 2 + ya:H, 4:5])
                        nc.scalar.copy(dst[0:P, 2 + ya:H, 1:2],
                                       dst[0:P, 2 + ya:H, 3:4])
                return scl, bia

            # ---- lift layer ----
            def lift_mms(ps, y0, rr):
                nc.tensor.matmul(ps[:], lw1[:],
                                 xc1[:, y0 * E0:(y0 + rr) * E0],
                                 start=True, stop=False)
                nc.tensor.matmul(ps[:], lw2[:],
                                 xc2[:, y0 * E0:(y0 + rr) * E0],
                                 start=False, stop=True)

            emit_layer(lift_mms, bufA, 2, E0, 80, Gt,
                       gbt[:, 0, 0:1], gbt[:, 0, 1:2], True)

            # ---- conv layers 1..9 ----
            buf = bufA
            for L in range(9):
                E_out = E0 - 2 * (L + 1)
                src = buf
                buf = bufB if src is bufA else bufA
                doff = 2 if L < 8 else 0

                def conv_mms(ps, y0, rr, L=L, src=src, E_out=E_out):
                    for d in range(25):
                        dy, dx = d // 5, d % 5
                        nc.tensor.matmul(
                            ps[:], cw[L][:, d, :],
                            src[:, y0 + dy:y0 + dy + rr, dx:dx + E_out],
                            start=(d == 0), stop=(d == 24))

                emit_layer(conv_mms, buf, doff, E_out, 80, Gt,
                           gbt[:, L + 1, 0:1], gbt[:, L + 1, 1:2], L < 8)

            # ---- conv10 (1x1, 80 -> 128 partitions) ----
            act10 = tailsp.tile([128, 64, 64], F16, tag="act10")

            def c10_mms(ps, y0, rr, src=buf):
                nc.tensor.matmul(ps[:], w10t[:], src[:, y0:y0 + rr, 0:64],
                                 start=True, stop=True)

            # ---- max over orientations + final 1x1 conv + sigmoid ----
            # DVE operands must be partition-aligned: realign the upper half
            # at each tree level via SBUF->SBUF DMA, then max lane-wise.
            outsb = tailsp.tile([1, 4096], F32, tag="outsb")
            mtiles = {}

            def emit_maxtree(src_ap):
                b0 = tailsp.tile([64, 4096], F16, tag="b0")
                nc.scalar.dma_start(out=b0[:], in_=src_ap[64:128, :])
                m1 = tailsp.tile([64, 4096], F16, tag="m1")
                nc.vector.tensor_max(m1[:], src_ap[0:64, :], b0[:])
                b1 = tailsp.tile([32, 4096], F16, tag="b1")
                nc.scalar.dma_start(out=b1[:], in_=m1[32:64, :])
                m2 = tailsp.tile([32, 4096], F16, tag="m2")
                nc.vector.tensor_max(m2[:], m1[0:32, :], b1[:])
                m2b = tailsp.tile([16, 4096], F16, tag="m2b")
                nc.scalar.dma_start(out=m2b[:], in_=m2[16:32, :])
                m3 = tailsp.tile([16, 4096], F16, tag="m3")
                nc.vector.tensor_max(m3[:], m2[0:16, :], m2b[:])
                mtiles['m3'] = m3

            a10 = act10[:].rearrange("p a b -> p (a b)")
            if fold_max:
                # gamma10 > 0 (host-verified): the BN affine + relu commute
                # with the orientation max, so run the max tree on the RAW
                # conv10 output -- it overlaps the layer's AllGather -- and
                # apply relu(s*h+b) on the reduced [16, 4096] afterwards.
                scl10, bia10 = emit_layer(
                    c10_mms, act10, 0, 64, 128, G10t,
                    gb10t[:, 0:1], gb10t[:, 1:2], False,
                    tail_fn=lambda: emit_maxtree(a10), do_apply=False)
                m3 = mtiles['m3']
                nc.scalar.activation(m3[:], m3[:], AF.Relu,
                                     bias=bia10[0:16, :], scale=scl10[0:16, :])
            else:
                emit_layer(c10_mms, act10, 0, 64, 128, G10t,
                           gb10t[:, 0:1], gb10t[:, 1:2], False)
                emit_maxtree(a10)
                m3 = mtiles['m3']
            for j in range(8):
                fp = psst.tile([1, 512], F32, tag="st")
                nc.tensor.matmul(fp[:], wfint[:],
                                 m3[:, 512 * j:512 * (j + 1)],
                                 start=True, stop=True)
                nc.scalar.activation(outsb[:, 512 * j:512 * (j + 1)],
                                     fp[:], AF.Sigmoid)
            nc.sync.dma_start(out=d_out.ap(), in_=outsb[:])

    nc.compile()
    return nc


_CACHED = {}


def _get_program(fold_max):
    key = ('nc', fold_max)
    if key not in _CACHED:
        _CACHED[key] = build_program(fold_max=fold_max)
    return _CACHED[key]


LAST = None


def kernel(**inputs):
    global LAST
    in_maps, cores = prep_in_maps(inputs)
    fold_max = bool((np.asarray(inputs['g10'], np.float32) > 0).all())
    nc = _get_program(fold_max)
    res = bass_utils.run_bass_kernel_spmd(
        nc, in_maps, core_ids=list(range(8)),
        trace=bool(os.environ.get("GK_TRACE")))
    LAST = res
    out = np.zeros((2, 1, 128, 128), np.float32)
    for (core, omap) in zip(cores, res.results):
        b, qy, qx = core
        q = np.asarray(omap['out'], np.float32).reshape(64, 64)
        if qy:
            q = q[::-1, :]
        if qx:
            q = q[:, ::-1]
        out[b, 0, qy * 64:(qy + 1) * 64, qx * 64:(qx + 1) * 64] = q
    return out


# revision 11
# speedup vs baseline: 1.0503x; 1.0503x over previous
"""Trainium2 Bass kernel for nn_BasicGroupCNN12 (SE(2) group CNN).

Strategy (8 NeuronCores):
  - Shard: 2 batch images x 4 spatial quadrants (64x64) = 8 cores.
  - Each core computes its quadrant with a shrinking redundant margin
    (lift needs +18, each 5x5 conv consumes 2) -> no halo exchange.
  - Host flips each quadrant (and the conv kernels) so every core runs the
    IDENTICAL program with the true image corner at local (0,0); reflect
    padding is then a uniform local copy on the low sides.
  - Training-mode BatchNorm needs global batch stats per layer: per-chunk
    bn_stats on the PSUM conv output (one DVE pass), bn_aggr to a local
    (mean, E[y^2]) pair, then an 8-core AllGather (floor ~5us vs ~10us for
    AllReduce) + local rank-sum + an orientation group-mean PE matmul.
    The margin chunks run on the PE while the collective is in flight.
  - Convs are per-offset matmuls: K=96 (ci*orient zero-padded from 80),
    M=80 (orient*co), N<=512 pixels, accumulated in PSUM over 25 offsets.
  - Activations live in two static ping-pong SBUF buffers [96,84,84];
    K-pad partitions 80:96 are zeroed once at startup (no per-layer memset).
  - Matmul operands fp16 (PSUM accumulation fp32): ~6e-4 final rel err.
"""

import os
import numpy as np

import concourse.bass as bass
import concourse.mybir as mybir
import concourse.tile as tile
from concourse import bacc
from concourse import bass_utils

F16 = mybir.dt.float16
F32 = mybir.dt.float32
NPDT = np.float16

O = 8        # orientations
C = 10       # channels per orientation (conv layers)
CF = 16      # channels in layer 10
E0 = 82      # lift output extent per core
EPS = 1e-5
QN = 64 * 64                 # in-image pixels per core
AF = mybir.ActivationFunctionType
ALU = mybir.AluOpType

# conv_i (i=1..9) output extents per core
CONV_E = [64 + 2 * (9 - i) for i in range(1, 10)]   # 80,78,...,64


# ---------------------------------------------------------------------------
# Host-side weight/input preparation (pure numpy)
# ---------------------------------------------------------------------------

def _rot_matrices(k, n):
    c = (k - 1) / 2.0
    mats = np.zeros((n, k * k, k * k), np.float32)
    for m in range(n):
        th = 2.0 * np.pi * m / n
        co, si = np.cos(th), np.sin(th)
        for i in range(k):
            for j in range(k):
                di, dj = i - c, j - c
                sy = co * di + si * dj + c
                sx = -si * di + co * dj + c
                i0, j0 = int(np.floor(sy)), int(np.floor(sx))
                fy, fx = sy - i0, sx - j0
                for ii, jj, w in ((i0, j0, (1 - fy) * (1 - fx)), (i0, j0 + 1, (1 - fy) * fx),
                                  (i0 + 1, j0, fy * (1 - fx)), (i0 + 1, j0 + 1, fy * fx)):
                    if 0 <= ii < k and 0 <= jj < k and w > 1e-8:
                        mats[m, i * k + j, ii * k + jj] = w
    return mats


def _prep_rotated_weights(lift_w, conv_w, w10):
    M7 = _rot_matrices(7, O)
    M5 = _rot_matrices(5, O)
    Wlift = np.zeros((O, C, 3, 7, 7), np.float32)          # [m,co,ci,dy,dx]
    for m in range(O):
        Wlift[m] = (lift_w.reshape(C, 3, 49) @ M7[m].T).reshape(C, 3, 7, 7)
    Wconv = []
    for L in range(9):
        W = conv_w[L]                                      # [co,ci,n,5,5]
        Wf = np.zeros((O, C, O, C, 5, 5), np.float32)      # [m,co,n,ci,dy,dx]
        for m in range(O):
            Wm = np.roll(W, m, axis=2)
            Wm = (Wm.reshape(C, C, O, 25) @ M5[m].T).reshape(C, C, O, 5, 5)
            Wf[m] = Wm.transpose(0, 2, 1, 3, 4)
        Wconv.append(Wf)
    # w10 [16,10,8,1,1] -> mat [K=n*10+ci, M=m*16+co]; K zero-padded to 96
    # (K=80 matmuls stream at half rate on TRN2; K>=96 at full rate)
    W10mat = np.zeros((96, 128), np.float32)
    for m in range(O):
        Wm = np.roll(w10[:, :, :, 0, 0], m, axis=2)        # [co,ci,n]
        W10mat[:80, m * CF:(m + 1) * CF] = Wm.transpose(2, 1, 0).reshape(80, CF)
    return Wlift, Wconv, W10mat


def _group_mat(P, nch):
    # orientation group-MEAN matrix: the collective carries per-partition
    # (mean, E[y^2]) local averages; merging 8 orientation partitions x 8
    # cores (all with equal counts) is a plain average -> 1/64.
    idx = np.arange(P)
    Gm = (idx[:, None] % nch == idx[None, :] % nch).astype(np.float32)
    return Gm * (1.0 / 64.0)


def prep_in_maps(inputs):
    x = np.asarray(inputs['x'], np.float32)
    lift_w = np.asarray(inputs['lift_w'], np.float32)
    conv_w = np.asarray(inputs['conv_w'], np.float32)
    w10 = np.asarray(inputs['w10'], np.float32)
    wfinal = np.asarray(inputs['wfinal'], np.float32)

    Wlift, Wconv, W10mat = _prep_rotated_weights(lift_w, conv_w, w10)

    gb = np.zeros((80, 10, 2), np.float32)
    gb[:, 0, 0] = np.asarray(inputs['lift_g'], np.float32)[np.arange(80) % C]
    gb[:, 0, 1] = np.asarray(inputs['lift_b'], np.float32)[np.arange(80) % C]
    cg = np.asarray(inputs['conv_g'], np.float32)
    cb = np.asarray(inputs['conv_b'], np.float32)
    for L in range(9):
        gb[:, L + 1, 0] = cg[L][np.arange(80) % C]
        gb[:, L + 1, 1] = cb[L][np.arange(80) % C]
    gb10 = np.zeros((128, 2), np.float32)
    gb10[:, 0] = np.asarray(inputs['g10'], np.float32)[np.arange(128) % CF]
    gb10[:, 1] = np.asarray(inputs['b10'], np.float32)[np.arange(128) % CF]

    G = _group_mat(80, C)
    G10 = _group_mat(128, CF)
    wfin = wfinal[0, :, 0, 0].reshape(16, 1).astype(NPDT)

    # per-(qy,qx) flipped weight variants
    wvar = {}
    for qy in range(2):
        for qx in range(2):
            fy = slice(None, None, -1) if qy else slice(None)
            fx = slice(None, None, -1) if qx else slice(None)
            Wl = Wlift[:, :, :, fy, fx]                    # [m,co,ci,dy,dx]
            liftWmat = np.zeros((192, 80), np.float32)     # 147 rows + K-pad
            liftWmat[:147] = (
                Wl.transpose(3, 4, 2, 0, 1).reshape(49, 3, 80).reshape(147, 80))
            Wc = np.zeros((96, 9, 25, 80), np.float32)     # [K(padded), L, d, M]
            for L in range(9):
                Wf = Wconv[L][:, :, :, :, fy, fx]          # [m,co,n,ci,dy,dx]
                Wd = Wf.transpose(4, 5, 2, 3, 0, 1).reshape(25, 80, 80)
                Wc[:80, L] = Wd.transpose(1, 0, 2)         # [K, d, M]
            wvar[(qy, qx)] = (liftWmat.astype(NPDT), Wc.astype(NPDT))

    in_maps = []
    cores = []
    for b in range(2):
        for qy in range(2):
            for qx in range(2):
                cores.append((b, qy, qx))
                xf = x[b]
                if qy:
                    xf = xf[:, ::-1, :]
                if qx:
                    xf = xf[:, :, ::-1]
                win = xf[:, 0:85, 0:85]
                xw = np.pad(win, ((0, 0), (3, 0), (3, 0)), mode='reflect')
                col = np.zeros((192, E0 * E0), NPDT)       # 147 rows + K-pad
                for t in range(49):
                    dy, dx = t // 7, t % 7
                    col[t * 3:t * 3 + 3] = (
                        xw[:, dy:dy + E0, dx:dx + E0].reshape(3, -1))
                liftWmat, Wc = wvar[(qy, qx)]
                in_maps.append({
                    'xcol1': np.ascontiguousarray(col[:96]),
                    'xcol2': np.ascontiguousarray(col[96:]),
                    'liftW1': np.ascontiguousarray(liftWmat[:96]),
                    'liftW2': np.ascontiguousarray(liftWmat[96:]),
                    'convW': Wc,
                    'w10': W10mat.astype(NPDT),
                    'wfin': wfin,
                    'G': G,
                    'G10': G10,
                    'gb': gb,
                    'gb10': gb10,
                })
    return in_maps, cores


# ---------------------------------------------------------------------------
# Bass program
# ---------------------------------------------------------------------------

def build_program(no_cc=None, nconv=None, stage=None, fold_max=True):
    # bisection knobs (env override): GK_NO_CC=1 replaces the AllGather with
    # local DRAM->DRAM copies; GK_NCONV=k emits only the first k conv layers;
    # GK_STAGE stops the program early.
    if no_cc is None:
        no_cc = bool(os.environ.get("GK_NO_CC"))
    if nconv is None:
        nconv = int(os.environ.get("GK_NCONV", "9"))
    if stage is None:
        stage = int(os.environ.get("GK_STAGE", "6"))
    nc = bacc.Bacc("TRN2", target_bir_lowering=False, debug=False,
                   enable_asserts=False, num_devices=8)

    d_xcol1 = nc.dram_tensor("xcol1", [96, E0 * E0], F16, kind="ExternalInput")
    d_xcol2 = nc.dram_tensor("xcol2", [96, E0 * E0], F16, kind="ExternalInput")
    d_liftW1 = nc.dram_tensor("liftW1", [96, 80], F16, kind="ExternalInput")
    d_liftW2 = nc.dram_tensor("liftW2", [96, 80], F16, kind="ExternalInput")
    d_convW = nc.dram_tensor("convW", [96, 9, 25, 80], F16, kind="ExternalInput")
    d_w10 = nc.dram_tensor("w10", [96, 128], F16, kind="ExternalInput")
    d_wfin = nc.dram_tensor("wfin", [16, 1], F16, kind="ExternalInput")
    d_G = nc.dram_tensor("G", [80, 80], F32, kind="ExternalInput")
    d_G10 = nc.dram_tensor("G10", [128, 128], F32, kind="ExternalInput")
    d_gb = nc.dram_tensor("gb", [80, 10, 2], F32, kind="ExternalInput")
    d_gb10 = nc.dram_tensor("gb10", [128, 2], F32, kind="ExternalInput")
    d_out = nc.dram_tensor("out", [1, 64 * 64], F32, kind="ExternalOutput")

    rg = [list(range(8))]

    with tile.TileContext(nc) as tc:
        with (
            tc.tile_pool(name="const", bufs=1) as constp,
            tc.tile_pool(name="tails", bufs=1) as tailsp,
            tc.tile_pool(name="smallp", bufs=3) as smallp,
            tc.tile_pool(name="pscv", bufs=6, space="PSUM") as pscv,
            tc.tile_pool(name="psst", bufs=2, space="PSUM") as psst,
            tc.tile_pool(name="dram", bufs=2, space="DRAM") as dramp,
        ):
            # ---- load constants ----
            xc1 = constp.tile([96, E0 * E0], F16, tag="xc1")
            nc.sync.dma_start(out=xc1[:], in_=d_xcol1.ap())
            xc2 = constp.tile([96, E0 * E0], F16, tag="xc2")
            nc.sync.dma_start(out=xc2[:], in_=d_xcol2.ap())
            lw1 = constp.tile([96, 80], F16, tag="lw1")
            nc.sync.dma_start(out=lw1[:], in_=d_liftW1.ap())
            lw2 = constp.tile([96, 80], F16, tag="lw2")
            nc.sync.dma_start(out=lw2[:], in_=d_liftW2.ap())
            cw = []
            for L in range(9):
                t = constp.tile([96, 25, 80], F16, tag=f"cw{L}")
                nc.sync.dma_start(out=t[:], in_=d_convW.ap()[:, L, :, :])
                cw.append(t)
            w10t = constp.tile([96, 128], F16, tag="w10")
            nc.sync.dma_start(out=w10t[:], in_=d_w10.ap())
            wfint = constp.tile([16, 1], F16, tag="wfin")
            nc.sync.dma_start(out=wfint[:], in_=d_wfin.ap())
            Gt = constp.tile([80, 80], F32, tag="G")
            nc.sync.dma_start(out=Gt[:], in_=d_G.ap())
            G10t = constp.tile([128, 128], F32, tag="G10")
            nc.sync.dma_start(out=G10t[:], in_=d_G10.ap())
            gbt = constp.tile([80, 10, 2], F32, tag="gb")
            nc.sync.dma_start(out=gbt[:], in_=d_gb.ap())
            gb10t = constp.tile([128, 2], F32, tag="gb10")
            nc.sync.dma_start(out=gb10t[:], in_=d_gb10.ap())
            epst = constp.tile([128, 1], F32, tag="eps")
            nc.vector.memset(epst[:], EPS)

            # ---- static ping-pong activation buffers; K-pad partitions
            # 80:96 hold zeros for the whole program (written once).
            bufA = constp.tile([96, 84, 84], F16, tag="bufA")
            bufB = constp.tile([96, 84, 84], F16, tag="bufB")
            nc.vector.memset(bufA[64:96, :, :], 0.0)
            nc.vector.memset(bufB[64:96, :, :], 0.0)

            def emit_collective(cin, cout):
                if no_cc:
                    for r in range(8):
                        nc.sync.dma_start(out=cout[r, :, :], in_=cin[:])
                else:
                    nc.gpsimd.collective_compute(
                        "AllGather", ALU.bypass, replica_groups=rg,
                        ins=[cin[:].opt()], outs=[cout[:].opt()])

            # warm-up collective (absorbs ncfw first-call latency)
            wu_in = dramp.tile([8, 1], F32, tag="cc_in")
            wu_out = dramp.tile([8, 8, 1], F32, tag="cc_out")
            nc.sync.dma_start(out=wu_in[:], in_=epst[0:8, 0:1])
            emit_collective(wu_in, wu_out)

            def emit_layer(mms_fn, dst, doff, E_out, P, Gtile, gamma_ap,
                           beta_ap, do_reflect, tail_fn=None, do_apply=True):
                """One conv-like layer with overlapped BN.

                Per img chunk: matmuls into PSUM, ACT copy PSUM->SBUF, and a
                one-pass DVE bn_stats on the owned region of the PSUM tile.
                bn_aggr merges the row 6-tuples into local (mean, var); the
                (mean, E[y^2]) pair rides an 8-core AllGather whose latency
                the margin chunks hide; post-collective: rank-sum, group-mean
                matmul, scale/bias chain, relu-apply in two row pieces.
                """
                r = 512 // E_out
                chunks = [(y0, min(r, E_out - y0)) for y0 in range(0, E_out, r)]
                img_chunks = [c for c in chunks if c[0] < 64]
                mar_chunks = [c for c in chunks if c[0] >= 64]
                bst = smallp.tile([P, 64, 6], F32, tag="bst")

                def do_chunk(y0, rr, stats):
                    ps = pscv.tile([P, rr, E_out], F32, tag="cv")
                    mms_fn(ps, y0, rr)
                    nc.scalar.copy(
                        dst[0:P, doff + y0:doff + y0 + rr,
                            doff:doff + E_out], ps[:])
                    if stats:
                        h = min(y0 + rr, 64) - y0
                        for k in range(h):
                            nc.vector.bn_stats(bst[:, y0 + k, :],
                                               ps[0:P, k, 0:64])

                for (y0, rr) in img_chunks:
                    do_chunk(y0, rr, True)
                # local (mean, var) over the owned 64x64, then (mean, E[y^2])
                with tc.high_priority():
                    mv = smallp.tile([P, 2], F32, tag="mv")
                    nc.vector.bn_aggr(
                        mv[:], bst[:].rearrange("p a b -> p (a b)"))
                    v = smallp.tile([P, 2], F32, tag="v")
                    nc.vector.tensor_copy(v[:, 0:1], mv[:, 0:1])
                    msq0 = smallp.tile([P, 1], F32, tag="msq0")
                    nc.vector.tensor_mul(msq0[:], mv[:, 0:1], mv[:, 0:1])
                    nc.vector.tensor_add(v[:, 1:2], msq0[:], mv[:, 1:2])
                    cin = dramp.tile([P, 2], F32, tag="cc_in")
                    cout = dramp.tile([8, P, 2], F32, tag="cc_out")
                    nc.sync.dma_start(out=cin[:], in_=v[:])
                    emit_collective(cin, cout)
                # margin chunks run on the PE while the AllGather is in flight
                for (y0, rr) in mar_chunks:
                    do_chunk(y0, rr, False)
                if tail_fn is not None:
                    tail_fn()
                # post-collective critical path at high priority so no
                # bulk work (apply piece B, next-layer copies) gets
                # scheduled ahead of it on any queue.
                with tc.high_priority():
                    post = smallp.tile([P, 2, 8], F32, tag="post")
                    nc.sync.dma_start(out=post[:],
                                      in_=cout[:].rearrange("r p s -> p s r"))
                    m8 = smallp.tile([P, 2], F32, tag="m8")
                    nc.vector.tensor_reduce(out=m8[:], in_=post[:],
                                            axis=mybir.AxisListType.X,
                                            op=ALU.add)
                    # group-mean over orientations+cores (Gtile = mask/64)
                    gps = psst.tile([P, 2], F32, tag="st")
                    nc.tensor.matmul(gps[:], Gtile[:], m8[:],
                                     start=True, stop=True)
                    # var = E[y^2] - mean^2
                    msq = smallp.tile([P, 1], F32, tag="msq")
                    nc.scalar.square(msq[:], gps[:, 0:1])
                    var = smallp.tile([P, 1], F32, tag="var")
                    nc.vector.tensor_sub(var[:], gps[:, 1:2], msq[:])
                    std = smallp.tile([P, 1], F32, tag="std")
                    nc.scalar.activation(std[:], var[:], AF.Sqrt,
                                         bias=epst[0:P, 0:1])
                    rstd = smallp.tile([P, 1], F32, tag="rstd")
                    nc.vector.reciprocal(rstd[:], std[:])
                    scl = smallp.tile([P, 1], F32, tag="scl")
                    nc.vector.tensor_mul(scl[:], rstd[:], gamma_ap)
                    tb = smallp.tile([P, 1], F32, tag="tb")
                    nc.vector.tensor_mul(tb[:], gps[:, 0:1], scl[:])
                    bia = smallp.tile([P, 1], F32, tag="bia")
                    nc.vector.tensor_sub(bia[:], beta_ap, tb[:])
                    if not do_apply:
                        return scl, bia
                    # apply relu(y*scl + bia) in place: piece A covers the
                    # rows the next layer's first chunk reads; its reflect
                    # copies run on the DVE (concurrent with piece B on ACT).
                    ya = min(12, E_out)
                    H = E_out + 2
                    pa = dst[0:P, doff:doff + ya, doff:doff + E_out]
                    nc.scalar.activation(pa, pa, AF.Relu, bias=bia[:],
                                         scale=scl[:])
                    if do_reflect:
                        nc.vector.tensor_copy(dst[0:P, 2:2 + ya, 0:1],
                                              dst[0:P, 2:2 + ya, 4:5])
                        nc.vector.tensor_copy(dst[0:P, 2:2 + ya, 1:2],
                                              dst[0:P, 2:2 + ya, 3:4])
                        nc.vector.tensor_copy(dst[0:P, 0:1, 0:H],
                                              dst[0:P, 4:5, 0:H])
                        nc.vector.tensor_copy(dst[0:P, 1:2, 0:H],
                                              dst[0:P, 3:4, 0:H])
                if ya < E_out:
                    pb = dst[0:P, doff + ya:doff + E_out, doff:doff + E_out]
                    nc.scalar.activation(pb, pb, AF.Relu, bias=bia[:],
                                         scale=scl[:])
                    if do_reflect:
                        nc.vector.tensor_copy(dst[0:P, 2 + ya:H, 0:1],
                                              dst[0:P, 2 + ya:H, 4:5])
                        nc.vector.tensor_copy(dst[0:P, 2 + ya:H, 1:2],
                                              dst[0:P, 2 + ya:H, 3:4])
                return scl, bia

            # ---- lift layer ----
            def lift_mms(ps, y0, rr):
                nc.tensor.matmul(ps[:], lw1[:],
                                 xc1[:, y0 * E0:(y0 + rr) * E0],
                                 start=True, stop=False)
                nc.tensor.matmul(ps[:], lw2[:],
                                 xc2[:, y0 * E0:(y0 + rr) * E0],
                                 start=False, stop=True)

            emit_layer(lift_mms, bufA, 2, E0, 80, Gt,
                       gbt[:, 0, 0:1], gbt[:, 0, 1:2], True)

            # ---- conv layers 1..9 ----
            buf = bufA
            for L in range(9):
                E_out = E0 - 2 * (L + 1)
                src = buf
                buf = bufB if src is bufA else bufA
                doff = 2 if L < 8 else 0

                def conv_mms(ps, y0, rr, L=L, src=src, E_out=E_out):
                    for d in range(25):
                        dy, dx = d // 5, d % 5
                        nc.tensor.matmul(
                            ps[:], cw[L][:, d, :],
                            src[:, y0 + dy:y0 + dy + rr, dx:dx + E_out],
                            start=(d == 0), stop=(d == 24))

                emit_layer(conv_mms, buf, doff, E_out, 80, Gt,
                           gbt[:, L + 1, 0:1], gbt[:, L + 1, 1:2], L < 8)

            # ---- conv10 (1x1, 80 -> 128 partitions) ----
            act10 = tailsp.tile([128, 64, 64], F16, tag="act10")

            def c10_mms(ps, y0, rr, src=buf):
                nc.tensor.matmul(ps[:], w10t[:], src[:, y0:y0 + rr, 0:64],
                                 start=True, stop=True)

            # ---- max over orientations + final 1x1 conv + sigmoid ----
            # DVE operands must be partition-aligned: realign the upper half
            # at each tree level via SBUF->SBUF DMA, then max lane-wise.
            outsb = tailsp.tile([1, 4096], F32, tag="outsb")
            mtiles = {}

            def emit_maxtree(src_ap):
                b0 = tailsp.tile([64, 4096], F16, tag="b0")
                nc.scalar.dma_start(out=b0[:], in_=src_ap[64:128, :])
                m1 = tailsp.tile([64, 4096], F16, tag="m1")
                nc.vector.tensor_max(m1[:], src_ap[0:64, :], b0[:])
                b1 = tailsp.tile([32, 4096], F16, tag="b1")
                nc.scalar.dma_start(out=b1[:], in_=m1[32:64, :])
                m2 = tailsp.tile([32, 4096], F16, tag="m2")
                nc.vector.tensor_max(m2[:], m1[0:32, :], b1[:])
                m2b = tailsp.tile([16, 4096], F16, tag="m2b")
                nc.scalar.dma_start(out=m2b[:], in_=m2[16:32, :])
                m3 = tailsp.tile([16, 4096], F16, tag="m3")
                nc.vector.tensor_max(m3[:], m2[0:16, :], m2b[:])
                mtiles['m3'] = m3

            a10 = act10[:].rearrange("p a b -> p (a b)")
            if fold_max:
                # gamma10 > 0 (host-verified): the BN affine + relu commute
                # with the orientation max, so run the max tree on the RAW
                # conv10 output -- it overlaps the layer's AllGather -- and
                # apply relu(s*h+b) on the reduced [16, 4096] afterwards.
                scl10, bia10 = emit_layer(
                    c10_mms, act10, 0, 64, 128, G10t,
                    gb10t[:, 0:1], gb10t[:, 1:2], False,
                    tail_fn=lambda: emit_maxtree(a10), do_apply=False)
                m3 = mtiles['m3']
                nc.scalar.activation(m3[:], m3[:], AF.Relu,
                                     bias=bia10[0:16, :], scale=scl10[0:16, :])
            else:
                emit_layer(c10_mms, act10, 0, 64, 128, G10t,
                           gb10t[:, 0:1], gb10t[:, 1:2], False)
                emit_maxtree(a10)
                m3 = mtiles['m3']
            for j in range(8):
                fp = psst.tile([1, 512], F32, tag="st")
                nc.tensor.matmul(fp[:], wfint[:],
                                 m3[:, 512 * j:512 * (j + 1)],
                                 start=True, stop=True)
                nc.scalar.activation(outsb[:, 512 * j:512 * (j + 1)],
                                     fp[:], AF.Sigmoid)
            nc.sync.dma_start(out=d_out.ap(), in_=outsb[:])

    nc.compile()
    return nc


_CACHED = {}


def _get_program(fold_max):
    key = ('nc', fold_max)
    if key not in _CACHED:
        _CACHED[key] = build_program(fold_max=fold_max)
    return _CACHED[key]


LAST = None


def kernel(**inputs):
    global LAST
    in_maps, cores = prep_in_maps(inputs)
    fold_max = bool((np.asarray(inputs['g10'], np.float32) > 0).all())
    nc = _get_program(fold_max)
    res = bass_utils.run_bass_kernel_spmd(
        nc, in_maps, core_ids=list(range(8)),
        trace=bool(os.environ.get("GK_TRACE")))
    LAST = res
    out = np.zeros((2, 1, 128, 128), np.float32)
    for (core, omap) in zip(cores, res.results):
        b, qy, qx = core
        q = np.asarray(omap['out'], np.float32).reshape(64, 64)
        if qy:
            q = q[::-1, :]
        if qx:
            q = q[:, ::-1]
        out[b, 0, qy * 64:(qy + 1) * 64, qx * 64:(qx + 1) * 64] = q
    return out


# revision 14
# speedup vs baseline: 1.0668x; 1.0157x over previous
"""Trainium2 Bass kernel for nn_BasicGroupCNN12 (SE(2) group CNN).

Strategy (8 NeuronCores):
  - Shard: 2 batch images x 4 spatial quadrants (64x64) = 8 cores.
  - Each core computes its quadrant with a shrinking redundant margin
    (lift needs +18, each 5x5 conv consumes 2) -> no halo exchange.
  - Host flips each quadrant (and the conv kernels) so every core runs the
    IDENTICAL program with the true image corner at local (0,0); reflect
    padding is then a uniform local copy on the low sides.
  - Training-mode BatchNorm needs global batch stats per layer: per-chunk
    bn_stats on the PSUM conv output (one DVE pass), bn_aggr to a local
    (mean, E[y^2]) pair, then an 8-core AllGather (floor ~5us vs ~10us for
    AllReduce) + local rank-sum + an orientation group-mean PE matmul.
    The margin chunks run on the PE while the collective is in flight.
  - Convs are per-offset matmuls: K=96 (ci*orient zero-padded from 80),
    M=80 (orient*co), N<=512 pixels, accumulated in PSUM over 25 offsets.
  - Activations live in two static ping-pong SBUF buffers [96,84,84];
    K-pad partitions 80:96 are zeroed once at startup (no per-layer memset).
  - Matmul operands fp16 (PSUM accumulation fp32): ~6e-4 final rel err.
"""

import os
import numpy as np

import concourse.bass as bass
import concourse.mybir as mybir
import concourse.tile as tile
from concourse import bacc
from concourse import bass_utils

F16 = mybir.dt.float16
F32 = mybir.dt.float32
NPDT = np.float16

O = 8        # orientations
C = 10       # channels per orientation (conv layers)
CF = 16      # channels in layer 10
E0 = 82      # lift output extent per core
EPS = 1e-5
QN = 64 * 64                 # in-image pixels per core
AF = mybir.ActivationFunctionType
ALU = mybir.AluOpType

# conv_i (i=1..9) output extents per core
CONV_E = [64 + 2 * (9 - i) for i in range(1, 10)]   # 80,78,...,64


# ---------------------------------------------------------------------------
# Host-side weight/input preparation (pure numpy)
# ---------------------------------------------------------------------------

def _rot_matrices(k, n):
    c = (k - 1) / 2.0
    mats = np.zeros((n, k * k, k * k), np.float32)
    for m in range(n):
        th = 2.0 * np.pi * m / n
        co, si = np.cos(th), np.sin(th)
        for i in range(k):
            for j in range(k):
                di, dj = i - c, j - c
                sy = co * di + si * dj + c
                sx = -si * di + co * dj + c
                i0, j0 = int(np.floor(sy)), int(np.floor(sx))
                fy, fx = sy - i0, sx - j0
                for ii, jj, w in ((i0, j0, (1 - fy) * (1 - fx)), (i0, j0 + 1, (1 - fy) * fx),
                                  (i0 + 1, j0, fy * (1 - fx)), (i0 + 1, j0 + 1, fy * fx)):
                    if 0 <= ii < k and 0 <= jj < k and w > 1e-8:
                        mats[m, i * k + j, ii * k + jj] = w
    return mats


def _prep_rotated_weights(lift_w, conv_w, w10):
    M7 = _rot_matrices(7, O)
    M5 = _rot_matrices(5, O)
    Wlift = np.zeros((O, C, 3, 7, 7), np.float32)          # [m,co,ci,dy,dx]
    for m in range(O):
        Wlift[m] = (lift_w.reshape(C, 3, 49) @ M7[m].T).reshape(C, 3, 7, 7)
    Wconv = []
    for L in range(9):
        W = conv_w[L]                                      # [co,ci,n,5,5]
        Wf = np.zeros((O, C, O, C, 5, 5), np.float32)      # [m,co,n,ci,dy,dx]
        for m in range(O):
            Wm = np.roll(W, m, axis=2)
            Wm = (Wm.reshape(C, C, O, 25) @ M5[m].T).reshape(C, C, O, 5, 5)
            Wf[m] = Wm.transpose(0, 2, 1, 3, 4)
        Wconv.append(Wf)
    # w10 [16,10,8,1,1] -> mat [K=n*10+ci, M=m*16+co]; K zero-padded to 96
    # (K=80 matmuls stream at half rate on TRN2; K>=96 at full rate)
    W10mat = np.zeros((96, 128), np.float32)
    for m in range(O):
        Wm = np.roll(w10[:, :, :, 0, 0], m, axis=2)        # [co,ci,n]
        W10mat[:80, m * CF:(m + 1) * CF] = Wm.transpose(2, 1, 0).reshape(80, CF)
    return Wlift, Wconv, W10mat


def _group_mat(P, nch):
    # orientation group-MEAN matrix: the collective carries per-partition
    # (mean, E[y^2]) local averages; merging 8 orientation partitions x 8
    # cores (all with equal counts) is a plain average -> 1/64.
    idx = np.arange(P)
    Gm = (idx[:, None] % nch == idx[None, :] % nch).astype(np.float32)
    return Gm * (1.0 / 64.0)


def prep_in_maps(inputs):
    x = np.asarray(inputs['x'], np.float32)
    lift_w = np.asarray(inputs['lift_w'], np.float32)
    conv_w = np.asarray(inputs['conv_w'], np.float32)
    w10 = np.asarray(inputs['w10'], np.float32)
    wfinal = np.asarray(inputs['wfinal'], np.float32)

    Wlift, Wconv, W10mat = _prep_rotated_weights(lift_w, conv_w, w10)

    gb = np.zeros((80, 10, 2), np.float32)
    gb[:, 0, 0] = np.asarray(inputs['lift_g'], np.float32)[np.arange(80) % C]
    gb[:, 0, 1] = np.asarray(inputs['lift_b'], np.float32)[np.arange(80) % C]
    cg = np.asarray(inputs['conv_g'], np.float32)
    cb = np.asarray(inputs['conv_b'], np.float32)
    for L in range(9):
        gb[:, L + 1, 0] = cg[L][np.arange(80) % C]
        gb[:, L + 1, 1] = cb[L][np.arange(80) % C]
    gb10 = np.zeros((128, 2), np.float32)
    gb10[:, 0] = np.asarray(inputs['g10'], np.float32)[np.arange(128) % CF]
    gb10[:, 1] = np.asarray(inputs['b10'], np.float32)[np.arange(128) % CF]

    G = _group_mat(80, C)
    G10 = _group_mat(128, CF)
    wfin = wfinal[0, :, 0, 0].reshape(16, 1).astype(NPDT)

    # per-(qy,qx) flipped weight variants
    wvar = {}
    for qy in range(2):
        for qx in range(2):
            fy = slice(None, None, -1) if qy else slice(None)
            fx = slice(None, None, -1) if qx else slice(None)
            Wl = Wlift[:, :, :, fy, fx]                    # [m,co,ci,dy,dx]
            liftWmat = np.zeros((192, 80), np.float32)     # 147 rows + K-pad
            liftWmat[:147] = (
                Wl.transpose(3, 4, 2, 0, 1).reshape(49, 3, 80).reshape(147, 80))
            Wc = np.zeros((96, 9, 25, 80), np.float32)     # [K(padded), L, d, M]
            for L in range(9):
                Wf = Wconv[L][:, :, :, :, fy, fx]          # [m,co,n,ci,dy,dx]
                Wd = Wf.transpose(4, 5, 2, 3, 0, 1).reshape(25, 80, 80)
                Wc[:80, L] = Wd.transpose(1, 0, 2)         # [K, d, M]
            wvar[(qy, qx)] = (liftWmat.astype(NPDT), Wc.astype(NPDT))

    in_maps = []
    cores = []
    for b in range(2):
        for qy in range(2):
            for qx in range(2):
                cores.append((b, qy, qx))
                xf = x[b]
                if qy:
                    xf = xf[:, ::-1, :]
                if qx:
                    xf = xf[:, :, ::-1]
                win = xf[:, 0:85, 0:85]
                xw = np.pad(win, ((0, 0), (3, 0), (3, 0)), mode='reflect')
                col = np.zeros((192, E0 * E0), NPDT)       # 147 rows + K-pad
                for t in range(49):
                    dy, dx = t // 7, t % 7
                    col[t * 3:t * 3 + 3] = (
                        xw[:, dy:dy + E0, dx:dx + E0].reshape(3, -1))
                liftWmat, Wc = wvar[(qy, qx)]
                in_maps.append({
                    'xcol1': np.ascontiguousarray(col[:96]),
                    'xcol2': np.ascontiguousarray(col[96:]),
                    'liftW1': np.ascontiguousarray(liftWmat[:96]),
                    'liftW2': np.ascontiguousarray(liftWmat[96:]),
                    'convW': Wc,
                    'w10': W10mat.astype(NPDT),
                    'wfin': wfin,
                    'G': G,
                    'G10': G10,
                    'gb': gb,
                    'gb10': gb10,
                })
    return in_maps, cores


# ---------------------------------------------------------------------------
# Bass program
# ---------------------------------------------------------------------------

def build_program(no_cc=None, nconv=None, stage=None, fold_max=True):
    # bisection knobs (env override): GK_NO_CC=1 replaces the AllGather with
    # local DRAM->DRAM copies; GK_NCONV=k emits only the first k conv layers;
    # GK_STAGE stops the program early.
    if no_cc is None:
        no_cc = bool(os.environ.get("GK_NO_CC"))
    if nconv is None:
        nconv = int(os.environ.get("GK_NCONV", "9"))
    if stage is None:
        stage = int(os.environ.get("GK_STAGE", "6"))
    nc = bacc.Bacc("TRN2", target_bir_lowering=False, debug=False,
                   enable_asserts=False, num_devices=8)

    d_xcol1 = nc.dram_tensor("xcol1", [96, E0 * E0], F16, kind="ExternalInput")
    d_xcol2 = nc.dram_tensor("xcol2", [96, E0 * E0], F16, kind="ExternalInput")
    d_liftW1 = nc.dram_tensor("liftW1", [96, 80], F16, kind="ExternalInput")
    d_liftW2 = nc.dram_tensor("liftW2", [96, 80], F16, kind="ExternalInput")
    d_convW = nc.dram_tensor("convW", [96, 9, 25, 80], F16, kind="ExternalInput")
    d_w10 = nc.dram_tensor("w10", [96, 128], F16, kind="ExternalInput")
    d_wfin = nc.dram_tensor("wfin", [16, 1], F16, kind="ExternalInput")
    d_G = nc.dram_tensor("G", [80, 80], F32, kind="ExternalInput")
    d_G10 = nc.dram_tensor("G10", [128, 128], F32, kind="ExternalInput")
    d_gb = nc.dram_tensor("gb", [80, 10, 2], F32, kind="ExternalInput")
    d_gb10 = nc.dram_tensor("gb10", [128, 2], F32, kind="ExternalInput")
    d_out = nc.dram_tensor("out", [1, 64 * 64], F32, kind="ExternalOutput")

    rg = [list(range(8))]

    with tile.TileContext(nc) as tc:
        with (
            tc.tile_pool(name="const", bufs=1) as constp,
            tc.tile_pool(name="tails", bufs=1) as tailsp,
            tc.tile_pool(name="smallp", bufs=3) as smallp,
            tc.tile_pool(name="pscv", bufs=6, space="PSUM") as pscv,
            tc.tile_pool(name="psst", bufs=2, space="PSUM") as psst,
            tc.tile_pool(name="dram", bufs=2, space="DRAM") as dramp,
        ):
            # ---- load constants ----
            xc1 = constp.tile([96, E0 * E0], F16, tag="xc1")
            nc.sync.dma_start(out=xc1[:], in_=d_xcol1.ap())
            xc2 = constp.tile([96, E0 * E0], F16, tag="xc2")
            nc.sync.dma_start(out=xc2[:], in_=d_xcol2.ap())
            lw1 = constp.tile([96, 80], F16, tag="lw1")
            nc.sync.dma_start(out=lw1[:], in_=d_liftW1.ap())
            lw2 = constp.tile([96, 80], F16, tag="lw2")
            nc.sync.dma_start(out=lw2[:], in_=d_liftW2.ap())
            cw = []
            for L in range(9):
                t = constp.tile([96, 25, 80], F16, tag=f"cw{L}")
                nc.sync.dma_start(out=t[:], in_=d_convW.ap()[:, L, :, :])
                cw.append(t)
            w10t = constp.tile([96, 128], F16, tag="w10")
            nc.sync.dma_start(out=w10t[:], in_=d_w10.ap())
            wfint = constp.tile([16, 1], F16, tag="wfin")
            nc.sync.dma_start(out=wfint[:], in_=d_wfin.ap())
            Gt = constp.tile([80, 80], F32, tag="G")
            nc.sync.dma_start(out=Gt[:], in_=d_G.ap())
            G10t = constp.tile([128, 128], F32, tag="G10")
            nc.sync.dma_start(out=G10t[:], in_=d_G10.ap())
            gbt = constp.tile([80, 10, 2], F32, tag="gb")
            nc.sync.dma_start(out=gbt[:], in_=d_gb.ap())
            gb10t = constp.tile([128, 2], F32, tag="gb10")
            nc.sync.dma_start(out=gb10t[:], in_=d_gb10.ap())
            epst = constp.tile([128, 1], F32, tag="eps")
            nc.vector.memset(epst[:], EPS)
            # preload the sigmoid ACT table so the final layer doesn't pay
            # the ~1.3us lazy table load on the critical tail
            sgw = constp.tile([1, 1], F32, tag="sgw")
            nc.scalar.activation(sgw[:], epst[0:1, 0:1], AF.Sigmoid)

            # ---- static ping-pong activation buffers; K-pad partitions
            # 80:96 hold zeros for the whole program (written once).
            bufA = constp.tile([96, 84, 84], F16, tag="bufA")
            bufB = constp.tile([96, 84, 84], F16, tag="bufB")
            nc.vector.memset(bufA[64:96, :, :], 0.0)
            nc.vector.memset(bufB[64:96, :, :], 0.0)

            def emit_collective(cin, cout):
                if no_cc:
                    for r in range(8):
                        nc.sync.dma_start(out=cout[r, :, :], in_=cin[:])
                else:
                    nc.gpsimd.collective_compute(
                        "AllGather", ALU.bypass, replica_groups=rg,
                        ins=[cin[:].opt()], outs=[cout[:].opt()])

            # warm-up collective (absorbs ncfw first-call latency)
            wu_in = dramp.tile([8, 1], F32, tag="cc_in")
            wu_out = dramp.tile([8, 8, 1], F32, tag="cc_out")
            nc.sync.dma_start(out=wu_in[:], in_=epst[0:8, 0:1])
            emit_collective(wu_in, wu_out)

            def emit_layer(mms_fn, dst, doff, E_out, P, Gtile, gamma_ap,
                           beta_ap, do_reflect, tail_fn=None, do_apply=True,
                           chunk_cb=None):
                """One conv-like layer with overlapped BN.

                Per img chunk: matmuls into PSUM, ACT copy PSUM->SBUF, and a
                one-pass DVE bn_stats on the owned region of the PSUM tile.
                bn_aggr merges the row 6-tuples into local (mean, var); the
                (mean, E[y^2]) pair rides an 8-core AllGather whose latency
                the margin chunks hide; post-collective: rank-sum, group-mean
                matmul, scale/bias chain, relu-apply in two row pieces.
                """
                r = 512 // E_out
                chunks = [(y0, min(r, E_out - y0)) for y0 in range(0, E_out, r)]
                img_chunks = [c for c in chunks if c[0] < 64]
                mar_chunks = [c for c in chunks if c[0] >= 64]
                # E_out==64: PSUM rows are contiguous, so one bn_stats covers
                # a whole 512-element chunk (sim-safe 2D/2D); otherwise the
                # strided rows force one bn_stats per row.
                e64 = E_out == 64
                nst = len(img_chunks) if e64 else 64
                bst = smallp.tile([P, nst, 6], F32, tag="bst")

                def do_chunk(ci, y0, rr, stats):
                    ps = pscv.tile([P, rr, E_out], F32, tag="cv")
                    mms_fn(ps, y0, rr)
                    nc.scalar.copy(
                        dst[0:P, doff + y0:doff + y0 + rr,
                            doff:doff + E_out], ps[:])
                    if stats:
                        h = min(y0 + rr, 64) - y0
                        if e64:
                            nc.vector.bn_stats(
                                bst[:, ci, :],
                                ps[0:P, 0:h, 0:64].rearrange(
                                    "p a b -> p (a b)"))
                        else:
                            for k in range(h):
                                nc.vector.bn_stats(bst[:, y0 + k, :],
                                                   ps[0:P, k, 0:64])
                    if chunk_cb is not None:
                        chunk_cb(y0, rr)

                for ci, (y0, rr) in enumerate(img_chunks):
                    do_chunk(ci, y0, rr, True)
                # local (mean, var) over the owned 64x64, then (mean, E[y^2])
                with tc.high_priority():
                    mv = smallp.tile([P, 2], F32, tag="mv")
                    nc.vector.bn_aggr(
                        mv[:], bst[:].rearrange("p a b -> p (a b)"))
                    v = smallp.tile([P, 2], F32, tag="v")
                    nc.vector.tensor_copy(v[:, 0:1], mv[:, 0:1])
                    msq0 = smallp.tile([P, 1], F32, tag="msq0")
                    nc.vector.tensor_mul(msq0[:], mv[:, 0:1], mv[:, 0:1])
                    nc.vector.tensor_add(v[:, 1:2], msq0[:], mv[:, 1:2])
                    cin = dramp.tile([P, 2], F32, tag="cc_in")
                    cout = dramp.tile([8, P, 2], F32, tag="cc_out")
                    nc.sync.dma_start(out=cin[:], in_=v[:])
                    emit_collective(cin, cout)
                # margin chunks run on the PE while the AllGather is in flight
                for (y0, rr) in mar_chunks:
                    do_chunk(0, y0, rr, False)
                if tail_fn is not None:
                    tail_fn()
                # post-collective critical path at high priority so no
                # bulk work (apply piece B, next-layer copies) gets
                # scheduled ahead of it on any queue.
                with tc.high_priority():
                    post = smallp.tile([P, 2, 8], F32, tag="post")
                    nc.sync.dma_start(out=post[:],
                                      in_=cout[:].rearrange("r p s -> p s r"))
                    m8 = smallp.tile([P, 2], F32, tag="m8")
                    nc.vector.tensor_reduce(out=m8[:], in_=post[:],
                                            axis=mybir.AxisListType.X,
                                            op=ALU.add)
                # group-mean over orientations+cores (Gtile = mask/64); normal
                # priority so it sits after the margin (and pacing) matmuls in
                # the in-order PE queue rather than blocking its head.
                gps = psst.tile([P, 2], F32, tag="st")
                nc.tensor.matmul(gps[:], Gtile[:], m8[:],
                                 start=True, stop=True)
                with tc.high_priority():
                    # var = E[y^2] - mean^2
                    msq = smallp.tile([P, 1], F32, tag="msq")
                    nc.scalar.square(msq[:], gps[:, 0:1])
                    var = smallp.tile([P, 1], F32, tag="var")
                    nc.vector.tensor_sub(var[:], gps[:, 1:2], msq[:])
                    std = smallp.tile([P, 1], F32, tag="std")
                    nc.scalar.activation(std[:], var[:], AF.Sqrt,
                                         bias=epst[0:P, 0:1])
                    rstd = smallp.tile([P, 1], F32, tag="rstd")
                    nc.vector.reciprocal(rstd[:], std[:])
                    scl = smallp.tile([P, 1], F32, tag="scl")
                    nc.vector.tensor_mul(scl[:], rstd[:], gamma_ap)
                    tb = smallp.tile([P, 1], F32, tag="tb")
                    nc.vector.tensor_mul(tb[:], gps[:, 0:1], scl[:])
                    bia = smallp.tile([P, 1], F32, tag="bia")
                    nc.vector.tensor_sub(bia[:], beta_ap, tb[:])
                    if not do_apply:
                        return scl, bia
                    # apply relu(y*scl + bia) in place: piece A covers the
                    # rows the next layer's first chunk reads; its reflect
                    # copies run on the DVE (concurrent with piece B on ACT).
                    ya = min(12, E_out)
                    H = E_out + 2
                    pa = dst[0:P, doff:doff + ya, doff:doff + E_out]
                    nc.scalar.activation(pa, pa, AF.Relu, bias=bia[:],
                                         scale=scl[:])
                    if do_reflect:
                        nc.vector.tensor_copy(dst[0:P, 2:2 + ya, 0:1],
                                              dst[0:P, 2:2 + ya, 4:5])
                        nc.vector.tensor_copy(dst[0:P, 2:2 + ya, 1:2],
                                              dst[0:P, 2:2 + ya, 3:4])
                        nc.vector.tensor_copy(dst[0:P, 0:1, 0:H],
                                              dst[0:P, 4:5, 0:H])
                        nc.vector.tensor_copy(dst[0:P, 1:2, 0:H],
                                              dst[0:P, 3:4, 0:H])
                if ya < E_out:
                    pb = dst[0:P, doff + ya:doff + E_out, doff:doff + E_out]
                    nc.scalar.activation(pb, pb, AF.Relu, bias=bia[:],
                                         scale=scl[:])
                    if do_reflect:
                        nc.vector.tensor_copy(dst[0:P, 2 + ya:H, 0:1],
                                              dst[0:P, 2 + ya:H, 4:5])
                        nc.vector.tensor_copy(dst[0:P, 2 + ya:H, 1:2],
                                              dst[0:P, 2 + ya:H, 3:4])
                return scl, bia

            # ---- lift layer ----
            def lift_mms(ps, y0, rr):
                nc.tensor.matmul(ps[:], lw1[:],
                                 xc1[:, y0 * E0:(y0 + rr) * E0],
                                 start=True, stop=False)
                nc.tensor.matmul(ps[:], lw2[:],
                                 xc2[:, y0 * E0:(y0 + rr) * E0],
                                 start=False, stop=True)

            emit_layer(lift_mms, bufA, 2, E0, 80, Gt,
                       gbt[:, 0, 0:1], gbt[:, 0, 1:2], True)

            # ---- conv layers 1..9 ----
            buf = bufA
            for L in range(9):
                E_out = E0 - 2 * (L + 1)
                src = buf
                buf = bufB if src is bufA else bufA
                doff = 2 if L < 8 else 0

                def conv_mms(ps, y0, rr, L=L, src=src, E_out=E_out):
                    for d in range(25):
                        dy, dx = d // 5, d % 5
                        nc.tensor.matmul(
                            ps[:], cw[L][:, d, :],
                            src[:, y0 + dy:y0 + dy + rr, dx:dx + E_out],
                            start=(d == 0), stop=(d == 24))

                emit_layer(conv_mms, buf, doff, E_out, 80, Gt,
                           gbt[:, L + 1, 0:1], gbt[:, L + 1, 1:2], L < 8)

            # ---- conv10 (1x1, 80 -> 128 partitions) ----
            act10 = tailsp.tile([128, 64, 64], F16, tag="act10")

            def c10_mms(ps, y0, rr, src=buf):
                nc.tensor.matmul(ps[:], w10t[:], src[:, y0:y0 + rr, 0:64],
                                 start=True, stop=True)

            # ---- max over orientations + final 1x1 conv + sigmoid ----
            # DVE operands must be partition-aligned: realign the upper half
            # at each tree level via SBUF->SBUF DMA, then max lane-wise.
            # Level-0 DMAs go per chunk (they complete during the conv10
            # matmul phase, keeping the collective's input DMA uncontended);
            # the max levels are emitted level-ordered so the per-chunk DMA
            # completion latencies pipeline instead of serializing the DVE.
            outsb = tailsp.tile([1, 4096], F32, tag="outsb")
            a10 = act10[:].rearrange("p a b -> p (a b)")
            b0 = tailsp.tile([64, 4096], F16, tag="b0")
            m1 = tailsp.tile([64, 4096], F16, tag="m1")
            b1 = tailsp.tile([32, 4096], F16, tag="b1")
            m2 = tailsp.tile([32, 4096], F16, tag="m2")
            m2b = tailsp.tile([16, 4096], F16, tag="m2b")
            m3 = tailsp.tile([16, 4096], F16, tag="m3")

            def tree_dma0(y0, rr):
                c0, n = y0 * 64, rr * 64
                nc.scalar.dma_start(out=b0[:, c0:c0 + n],
                                    in_=a10[64:128, c0:c0 + n])

            def tree_levels():
                cols = [(y0 * 64, rr * 64)
                        for (y0, rr) in [(k * 8, 8) for k in range(8)]]
                for c0, n in cols:
                    nc.vector.tensor_max(m1[:, c0:c0 + n],
                                         a10[0:64, c0:c0 + n],
                                         b0[:, c0:c0 + n])
                    nc.scalar.dma_start(out=b1[:, c0:c0 + n],
                                        in_=m1[32:64, c0:c0 + n])
                for c0, n in cols:
                    nc.vector.tensor_max(m2[:, c0:c0 + n],
                                         m1[0:32, c0:c0 + n],
                                         b1[:, c0:c0 + n])
                    nc.scalar.dma_start(out=m2b[:, c0:c0 + n],
                                        in_=m2[16:32, c0:c0 + n])
                for c0, n in cols:
                    nc.vector.tensor_max(m3[:, c0:c0 + n],
                                         m2[0:16, c0:c0 + n],
                                         m2b[:, c0:c0 + n])

            if fold_max:
                # gamma10 > 0 (host-verified): the BN affine + relu commute
                # with the orientation max, so run the max tree on the RAW
                # conv10 output -- it overlaps the layer's AllGather -- and
                # apply relu on the reduced [16, 4096] with the BN scale
                # folded into the final conv weights:
                #   sigmoid(wfin . relu(s*h+b)) = sigmoid((wfin*s) . relu(h + b/s))
                scl10, bia10 = emit_layer(
                    c10_mms, act10, 0, 64, 128, G10t,
                    gb10t[:, 0:1], gb10t[:, 1:2], False,
                    tail_fn=tree_levels, do_apply=False,
                    chunk_cb=tree_dma0)
                rcp10 = tailsp.tile([16, 1], F32, tag="rcp10")
                nc.vector.reciprocal(rcp10[:], scl10[0:16, :])
                bia2 = tailsp.tile([16, 1], F32, tag="bia2")
                nc.vector.tensor_mul(bia2[:], bia10[0:16, :], rcp10[:])
                wfin2 = tailsp.tile([16, 1], F16, tag="wfin2")
                nc.vector.tensor_mul(wfin2[:], wfint[:], scl10[0:16, :])
                m3p = tailsp.tile([16, 4096], F16, tag="m3p")
                for j in range(8):
                    cj = slice(512 * j, 512 * (j + 1))
                    nc.vector.tensor_scalar(m3p[:, cj], m3[:, cj],
                                            bia2[:], 0.0, ALU.add, ALU.max)
                    fp = psst.tile([1, 512], F32, tag="st")
                    nc.tensor.matmul(fp[:], wfin2[:], m3p[:, cj],
                                     start=True, stop=True)
                    nc.scalar.activation(outsb[:, cj], fp[:], AF.Sigmoid)
            else:
                # tree DMAs must read the APPLIED values here, so they run
                # after the layer's in-place BN apply (no per-chunk overlap)
                emit_layer(c10_mms, act10, 0, 64, 128, G10t,
                           gb10t[:, 0:1], gb10t[:, 1:2], False)
                nc.scalar.dma_start(out=b0[:], in_=a10[64:128, :])
                tree_levels()
                for j in range(8):
                    cj = slice(512 * j, 512 * (j + 1))
                    fp = psst.tile([1, 512], F32, tag="st")
                    nc.tensor.matmul(fp[:], wfint[:], m3[:, cj],
                                     start=True, stop=True)
                    nc.scalar.activation(outsb[:, cj], fp[:], AF.Sigmoid)
            nc.sync.dma_start(out=d_out.ap(), in_=outsb[:])

    nc.compile()
    return nc


_CACHED = {}


def _get_program(fold_max):
    key = ('nc', fold_max)
    if key not in _CACHED:
        _CACHED[key] = build_program(fold_max=fold_max)
    return _CACHED[key]


LAST = None


def kernel(**inputs):
    global LAST
    in_maps, cores = prep_in_maps(inputs)
    fold_max = bool((np.asarray(inputs['g10'], np.float32) > 0).all())
    nc = _get_program(fold_max)
    res = bass_utils.run_bass_kernel_spmd(
        nc, in_maps, core_ids=list(range(8)),
        trace=bool(os.environ.get("GK_TRACE")))
    LAST = res
    out = np.zeros((2, 1, 128, 128), np.float32)
    for (core, omap) in zip(cores, res.results):
        b, qy, qx = core
        q = np.asarray(omap['out'], np.float32).reshape(64, 64)
        if qy:
            q = q[::-1, :]
        if qx:
            q = q[:, ::-1]
        out[b, 0, qy * 64:(qy + 1) * 64, qx * 64:(qx + 1) * 64] = q
    return out
